# revision 88
# baseline (speedup 1.0000x reference)
"""Trainium2 Bass kernel for nn_BlocksCore (topk_masking).

Contract: kernel(**inputs) takes FULL unsharded inputs (B=4096) and returns
(hx_out, cx_out, mask_w), each (4096, 2048) float32 — matching reference().

Strategy:
  - Pure data parallel over 8 NeuronCores: 512 batch rows per core;
    per-block weights replicated.
  - Host-side algebraic folding (verified to 1.9e-7 rel err vs reference):
      * read-slot 0 is all zeros => input attention softmax over 2 slots
        collapses to sig = sigmoid(q . k1 / 8)
      * fold W3 = Wv_i[1] @ fc_i_w @ Wih_cat  (512 x 6144) so the GRU x-gates
        become  gx[b,k,:] = sig[b,k] * (inp[b] @ W3)[k*768:(k+1)*768]
      * top-k drop mask == keep the 4 blocks with largest s (rank by count)
  - On-chip layout: batch-major (batch on partitions) for all pointwise work
    (sig/mask are then per-partition scalars -> fused scalar_tensor_tensor),
    feature-major stationary operands (via PE transpose) for matmuls.
  - dtypes: s-path (q, k1, dot) in exact fp32 (mask threshold gap ~1.5e-6);
    big tolerant matmuls (G, gh, mha qkv, att) in bf16.
"""

import os
import numpy as np

import concourse.bass as bass
import concourse.bacc as bacc
import concourse.tile as tile
import concourse.mybir as mybir
from concourse.masks import make_identity

# ---- problem constants (hardcoded per contract) ----
B_FULL = 4096
N_CORES = 8
B = B_FULL // N_CORES          # 512 per core
NG = B // 128                  # 4 groups of 128 batch rows per core
NINP = 512
NHID = 2048
NBO = 8
BSO = 256
TOPK = 4
DK_I = 64
NH_M, DK_M, DV_M = 4, 16, 16
G3 = 3 * BSO                   # 768 gate width per block
HD = NH_M * DK_M               # 64

f32 = mybir.dt.float32
bf16 = mybir.dt.bfloat16
AF = mybir.ActivationFunctionType
ALU = mybir.AluOpType
AX = mybir.AxisListType

_CACHE = {}
last_results = None  # BassKernelResults of the most recent HW run


def _ap(t, free_dims, offset_elems=0):
    """Custom AP over a tile's free space: partition dim kept from the tile,
    free_dims = [(step, count), ...] in elements of the tile's free layout."""
    base = t if isinstance(t, bass.AP) else t[:]
    ap = [list(base.ap[0])] + [[s, c] for (s, c) in free_dims]
    return bass.AP(tensor=base.tensor, offset=base.offset + offset_elems, ap=ap)


def build_program():
    """Build (and cache) the per-core Bass program."""
    if "nc" in _CACHE:
        return _CACHE["nc"]

    nc = bacc.Bacc("TRN2", target_bir_lowering=False, debug=False)

    # ---- DRAM I/O (names are the in_map keys) ----
    d_inp = nc.dram_tensor("inp", [B, NINP], f32, kind="ExternalInput")
    d_hx = nc.dram_tensor("hx", [B, NHID], f32, kind="ExternalInput")
    d_cx = nc.dram_tensor("cx", [B, NHID], f32, kind="ExternalInput")
    # weights pre-arranged on host into SBUF-ready layouts (contiguous DMA)
    d_w3 = nc.dram_tensor("w3", [128, 4, NBO * G3], bf16, kind="ExternalInput")
    d_whh = nc.dram_tensor("whh", [128, 2, NBO, G3], bf16, kind="ExternalInput")
    d_wqkv = nc.dram_tensor("wqkv", [128, 2, NBO, 3 * HD], bf16,
                            kind="ExternalInput")
    d_wfg = nc.dram_tensor("wfg", [HD, 2 * BSO], bf16, kind="ExternalInput")
    d_wq = nc.dram_tensor("wq", [DK_I, NBO, BSO], f32, kind="ExternalInput")
    d_wk1 = nc.dram_tensor("wk1", [128, 4, DK_I], f32, kind="ExternalInput")

    d_hxo = nc.dram_tensor("hx_out", [B, NHID], f32, kind="ExternalOutput")
    d_cxo = nc.dram_tensor("cx_out", [B, NHID], f32, kind="ExternalOutput")
    d_mw = nc.dram_tensor("mask_w", [B, NHID], f32, kind="ExternalOutput")

    with tile.TileContext(nc) as tc:
        with (
            tc.tile_pool(name="consts", bufs=1) as consts,
            tc.tile_pool(name="io", bufs=2) as io,
            tc.tile_pool(name="io1", bufs=1) as io1,
            tc.tile_pool(name="fm", bufs=1) as fm,
            tc.tile_pool(name="fm2", bufs=2) as fm2,
            tc.tile_pool(name="work", bufs=1) as work,
            tc.tile_pool(name="work2", bufs=2) as work2,
            tc.tile_pool(name="small", bufs=2) as small,
            tc.tile_pool(name="gru3", bufs=3) as gru3,
            # PSUM: 8 banks of (128 x 2KB); one deep pool of (128,512)f32
            # single-bank slots maximizes cross-phase overlap, plus one
            # 2-bank slot for the paired att matmuls.
            tc.tile_pool(name="ps_sm", bufs=6, space="PSUM") as ps_sm,
            tc.tile_pool(name="ps_fg", bufs=1, space="PSUM") as ps_fg,
        ):
            # ---- resident constants / weights ----
            ident = consts.tile([128, 128], f32)
            make_identity(nc, ident)
            ident_bf = consts.tile([128, 128], bf16)
            make_identity(nc, ident_bf)

            # w3/whh are large: allocate now, DMA after group 0's input
            # loads (w3 in per-block chunks) so group 0 isn't stuck behind
            # ~10MB of weight traffic.
            w3_sb = consts.tile([128, 4, NBO * G3], bf16)
            whh_sb = consts.tile([128, 2, NBO, G3], bf16)
            wqkv_sb = consts.tile([128, 2, NBO, 3 * HD], bf16)
            nc.sync.dma_start(out=wqkv_sb, in_=d_wqkv[:])
            wfg_sb = consts.tile([HD, 2 * BSO], bf16)
            nc.sync.dma_start(out=wfg_sb, in_=d_wfg[:])
            wq_sb = consts.tile([DK_I, NBO, BSO], f32)
            nc.sync.dma_start(out=wq_sb, in_=d_wq[:])
            wk1_sb = consts.tile([128, 4, DK_I], f32)
            nc.sync.dma_start(out=wk1_sb, in_=d_wk1[:])

            def front(g, st):
                """PE-heavy first half of a group: loads, transposes, s-path,
                GRU, h_new transpose, mha qkv.  Generator: yields at segment
                boundaries so the driver can interleave with back(g-1)."""
                rows = slice(g * 128, (g + 1) * 128)

                # ---- load batch-major activations ----
                inp_bm = io.tile([128, NINP], f32, tag="inp_bm")
                nc.sync.dma_start(out=inp_bm, in_=d_inp[rows, :])
                hx_bm = io.tile([128, NHID], f32, tag="hx_bm")
                nc.sync.dma_start(out=hx_bm, in_=d_hx[rows, :])
                if g == 0:
                    # big weights ride behind group 0's activations
                    for k in range(NBO):
                        csl = slice(k * G3, (k + 1) * G3)
                        nc.sync.dma_start(out=w3_sb[:, :, csl],
                                          in_=d_w3[:, :, csl])
                    nc.sync.dma_start(out=whh_sb, in_=d_whh[:])

                # ---- feature-major copies via PE transpose; 4 transposes per
                # PSUM bank, one batched evict each (fp32 for k1, bf16 else) --
                inp_fm = fm.tile([128, 4, 128], f32, tag="inp_fm")
                inp_fmb = fm2.tile([128, 4, 128], bf16, tag="inp_fmb")
                pt = ps_sm.tile([128, 512], f32, tag="sm")
                for c in range(4):
                    nc.tensor.transpose(pt[:, c * 128:(c + 1) * 128],
                                        inp_bm[:, c * 128:(c + 1) * 128], ident)
                nc.vector.tensor_copy(out=_ap(inp_fm, [(1, 512)]), in_=pt)
                nc.scalar.activation(_ap(inp_fmb, [(1, 512)]), pt, AF.Copy)
                hx_fmb = fm2.tile([128, 16, 128], bf16, tag="hx_fmb")
                for t in range(4):
                    pt = ps_sm.tile([128, 512], f32, tag="sm")
                    for c in range(4):
                        cc = t * 4 + c
                        nc.tensor.transpose(pt[:, c * 128:(c + 1) * 128],
                                            hx_bm[:, cc * 128:(cc + 1) * 128],
                                            ident)
                    if t % 2 == 0:
                        nc.vector.tensor_copy(
                            out=_ap(hx_fmb, [(1, 512)], offset_elems=t * 512),
                            in_=pt)
                    else:
                        nc.scalar.activation(
                            _ap(hx_fmb, [(1, 512)], offset_elems=t * 512),
                            pt, AF.Copy)

                # ---- s-path (exact fp32): s[b,n] = hx3[b,n] . (Wq_n @ k1[b])
                # (1/8 folded into wq on host) ----
                k1_ps = ps_sm.tile([128, DK_I], f32, tag="sm")
                for c in range(4):
                    nc.tensor.matmul(k1_ps, inp_fm[:, c, :], wk1_sb[:, c, :],
                                     start=(c == 0), stop=(c == 3))
                k1_sb = small.tile([128, DK_I], f32, tag="k1sb")
                nc.scalar.activation(k1_sb, k1_ps, AF.Copy)
                k1_fm = small.tile([DK_I, 128], f32, tag="k1fm")
                ptk = ps_sm.tile([128, 512], f32, tag="sm")
                nc.tensor.transpose(ptk[0:DK_I, 0:128], k1_sb, ident)
                nc.vector.tensor_copy(out=k1_fm, in_=ptk[0:DK_I, 0:128])
                s_sb = small.tile([128, NBO], f32, tag="s")
                for i in range(NBO // 2):
                    u_ps = ps_sm.tile([128, 2, BSO], f32, tag="sm")
                    for j in range(2):
                        n = 2 * i + j
                        nc.tensor.matmul(u_ps[:, j, :], k1_fm, wq_sb[:, n, :],
                                         start=True, stop=True)
                    for j in range(2):
                        n = 2 * i + j
                        sp = small.tile([128, BSO], f32, tag="rhn")
                        # fused multiply + full-free accumulate: s_n = sum_i
                        # hx3[b,n,i] * u[b,n,i]
                        nc.vector.scalar_tensor_tensor(
                            out=sp, in0=hx_bm[:, n * BSO:(n + 1) * BSO],
                            scalar=1.0, in1=u_ps[:, j, :],
                            op0=ALU.mult, op1=ALU.mult,
                            accum_out=s_sb[:, n:n + 1])
                yield
                sig = small.tile([128, NBO], f32, tag="sig")
                nc.scalar.activation(sig, s_sb, AF.Sigmoid)
                # mask: keep block n iff #{m: s_m < s_n} >= NBO - TOPK
                ltmat = small.tile([128, NBO, NBO], f32, tag="ltmat")
                nc.vector.tensor_tensor(
                    out=ltmat,
                    in0=_ap(s_sb, [(0, NBO), (1, NBO)]),   # [n, m] -> s_m
                    in1=_ap(s_sb, [(1, NBO), (0, NBO)]),   # [n, m] -> s_n
                    op=ALU.is_lt)
                cnt = small.tile([128, NBO], f32, tag="cnt")
                nc.vector.tensor_reduce(cnt, ltmat, axis=AX.X, op=ALU.add)
                mask = small.tile([128, NBO], f32, tag="mask")
                nc.vector.tensor_scalar(
                    out=mask, in0=cnt, scalar1=float(NBO - TOPK) - 0.5,
                    scalar2=None, op0=ALU.is_ge)

                # ---- GRU per block k.  Three 1-bank PSUM tiles per k (G_rz,
                # ghrz, G_n|hn) through one deep pool.  1-k software skew:
                # emit matmuls+evict for k before the pointwise of k-1 so the
                # ACT stream never puts evict(k) behind sigmoid/tanh(k-1). ----
                h_new = work2.tile([128, NHID], f32, tag="h_new")

                def gru_produce(k):
                    grz = ps_sm.tile([128, 512], f32, tag="sm", name="grz")
                    gnh = ps_sm.tile([128, 512], f32, tag="sm", name="gnh")
                    ghz = ps_sm.tile([128, 512], f32, tag="sm", name="ghz")
                    # gh matmuls first: the ACT evict can fire ~3us earlier
                    for c in range(2):
                        nc.tensor.matmul(ghz,
                                         hx_fmb[:, k * 2 + c, :],
                                         whh_sb[:, c, k, 0:512],
                                         start=(c == 0), stop=(c == 1))
                        nc.tensor.matmul(gnh[:, BSO:512],
                                         hx_fmb[:, k * 2 + c, :],
                                         whh_sb[:, c, k, 512:G3],
                                         start=(c == 0), stop=(c == 1))
                    ghrz_sb = gru3.tile([128, 512], f32, tag="ghrz_sb")
                    nc.scalar.activation(ghrz_sb, ghz, AF.Copy)
                    for c in range(4):
                        nc.tensor.matmul(
                            grz, inp_fmb[:, c, :],
                            w3_sb[:, c, k * G3:k * G3 + 512],
                            start=(c == 0), stop=(c == 3))
                        nc.tensor.matmul(
                            gnh[:, 0:BSO], inp_fmb[:, c, :],
                            w3_sb[:, c, k * G3 + 512:(k + 1) * G3],
                            start=(c == 0), stop=(c == 3))
                    return grz, gnh, ghrz_sb

                def gru_pointwise(k, grz, gnh, ghrz_sb):
                    ksl = slice(k * BSO, (k + 1) * BSO)
                    sig_k = sig[:, k:k + 1]
                    rz_arg = gru3.tile([128, 512], f32, tag="rz_arg")
                    nc.vector.scalar_tensor_tensor(
                        out=rz_arg, in0=grz, scalar=sig_k,
                        in1=ghrz_sb, op0=ALU.mult, op1=ALU.add)
                    rz = rz_arg
                    nc.scalar.activation(rz, rz_arg, AF.Sigmoid)
                    rhn = small.tile([128, BSO], f32, tag="rhn")
                    nc.vector.tensor_mul(rhn, rz[:, 0:BSO], gnh[:, BSO:512])
                    n_arg = gru3.tile([128, BSO], f32, tag="n_arg")
                    nc.vector.scalar_tensor_tensor(
                        out=n_arg, in0=gnh[:, 0:BSO], scalar=sig_k,
                        in1=rhn, op0=ALU.mult, op1=ALU.add)
                    n_sb = n_arg
                    nc.scalar.activation(n_sb, n_arg, AF.Tanh)
                    d_sb = gru3.tile([128, BSO], f32, tag="d_sb")
                    nc.gpsimd.tensor_sub(d_sb, hx_bm[:, ksl], n_sb)
                    zd = gru3.tile([128, BSO], f32, tag="zd")
                    nc.vector.tensor_mul(zd, rz[:, BSO:512], d_sb)
                    nc.gpsimd.tensor_add(h_new[:, ksl], n_sb, zd)

                # h_new -> feature-major bf16 transposes and the qkv matmuls
                # are folded into the GRU loop: pair t (blocks 2t, 2t+1) is
                # transposed and its qkv computed as soon as both blocks'
                # pointwise is emitted.
                hn_fmb = fm2.tile([128, 16, 128], bf16, tag="hn_fmb")
                qkv_sb = work2.tile([128, NBO, 3 * HD], bf16, tag="qkv")

                def hn_pair(t):
                    pt = ps_sm.tile([128, 512], f32, tag="sm")
                    for c in range(4):
                        cc = t * 4 + c
                        nc.tensor.transpose(pt[:, c * 128:(c + 1) * 128],
                                            h_new[:, cc * 128:(cc + 1) * 128],
                                            ident)
                    if t % 2 == 0:
                        nc.vector.tensor_copy(
                            out=_ap(hn_fmb, [(1, 512)], offset_elems=t * 512),
                            in_=pt)
                    else:
                        nc.scalar.activation(
                            _ap(hn_fmb, [(1, 512)], offset_elems=t * 512),
                            pt, AF.Copy)
                    qkv_ps = ps_sm.tile([128, 2, 3 * HD], f32, tag="sm")
                    for j in range(2):
                        n = 2 * t + j
                        for c in range(2):
                            nc.tensor.matmul(qkv_ps[:, j, :],
                                             hn_fmb[:, n * 2 + c, :],
                                             wqkv_sb[:, c, n, :],
                                             start=(c == 0), stop=(c == 1))
                    nc.scalar.activation(
                        _ap(qkv_sb, [(1, 2 * 3 * HD)],
                            offset_elems=2 * t * 3 * HD),
                        _ap(qkv_ps, [(1, 2 * 3 * HD)]), AF.Copy)

                pend = None
                for k in range(NBO):
                    if k in (3, 6):
                        yield
                    prod = gru_produce(k)
                    if pend is not None:
                        gru_pointwise(k - 1, *pend)
                    pend = prod
                gru_pointwise(NBO - 1, *pend)
                yield
                for t in range(4):
                    hn_pair(t)

                st.update(dict(g=g, rows=rows, hx_bm=hx_bm,
                               h_new=h_new, qkv_sb=qkv_sb, mask=mask))

            def back(st):
                """DVE-heavy second half: mha pointwise, att, blends, stores.
                Generator with yield points matching front()'s segments."""
                g, rows = st["g"], st["rows"]
                hx_bm = st["hx_bm"]
                h_new, qkv_sb, mask = st["h_new"], st["qkv_sb"], st["mask"]
                cx_bm = io1.tile([128, NHID], f32, tag="cx_bm")
                nc.sync.dma_start(out=cx_bm, in_=d_cx[rows, :])

                # logits: P[h,q,k,d] = qm[q,h,d]*km[k,h,d]; l = sum_d P
                # (HW ISA limit: max 3 free dims per op -> loop over heads)
                l_sb = work.tile([128, NH_M * NBO, NBO], bf16, tag="l")
                with nc.allow_low_precision("mha logits tolerate bf16"):
                    for h in range(NH_M):
                        P_h = work.tile([128, NBO, NBO, DK_M], bf16, tag="P",
                                        name="P_h")
                        nc.vector.tensor_mul(
                            P_h,
                            _ap(qkv_sb, [(3 * HD, NBO), (0, NBO), (1, DK_M)],
                                offset_elems=h * DK_M),
                            _ap(qkv_sb, [(0, NBO), (3 * HD, NBO), (1, DK_M)],
                                offset_elems=HD + h * DK_M))
                        nc.vector.tensor_reduce(
                            l_sb[:, h * NBO:(h + 1) * NBO, :], P_h,
                            axis=AX.X, op=ALU.add)
                yield
                e_sb = work.tile([128, NH_M * NBO, NBO], f32, tag="e")
                nc.scalar.activation(e_sb, l_sb, AF.Exp)
                esum = small.tile([128, NH_M * NBO], f32, tag="esum")
                nc.vector.tensor_reduce(esum, e_sb, axis=AX.X, op=ALU.add)
                erec = small.tile([128, NH_M * NBO], f32, tag="erec")
                nc.vector.reciprocal(erec, esum)
                am = work.tile([128, NH_M, NBO, NBO], bf16, tag="am")
                nc.vector.tensor_mul(
                    _ap(am, [(1, NH_M * NBO * NBO)]).rearrange(
                        "p (a k) -> p a k", k=NBO),
                    e_sb, _ap(erec, [(1, NH_M * NBO), (0, NBO)]))
                # om[q,h,d] = sum_k am[h,q,k] * vm[k,h,d]  (per-head, 3 free dims)
                om_bm = work.tile([128, NBO * HD], bf16, tag="om_bm")
                with nc.allow_low_precision("mha out tolerates bf16"):
                    for h in range(NH_M):
                        P2_h = work.tile([128, NBO, DK_M, NBO], bf16, tag="P",
                                         name="P2_h")
                        nc.gpsimd.tensor_mul(
                            P2_h,
                            _ap(am, [(NBO, NBO), (0, DK_M), (1, NBO)],
                                offset_elems=h * NBO * NBO),
                            _ap(qkv_sb, [(0, NBO), (1, DK_M), (3 * HD, NBO)],
                                offset_elems=2 * HD + h * DK_M))
                        nc.vector.tensor_reduce(
                            _ap(om_bm, [(HD, NBO), (1, DK_M)],
                                offset_elems=h * DK_M),
                            P2_h, axis=AX.X, op=ALU.add)

                yield
                # om -> feature-major bf16 (PE transpose: 64-wide chunks are
                # below the DMA xbar's 128-col minimum)
                om_fmb = fm.tile([HD, NBO, 128], bf16, tag="om_fmb")
                for t in range(2):
                    pt2 = ps_sm.tile([128, 512], bf16, tag="sm")
                    for c in range(4):
                        n = t * 4 + c
                        nc.tensor.transpose(
                            pt2[0:HD, c * 128:(c + 1) * 128],
                            om_bm[:, n * HD:(n + 1) * HD], ident_bf)
                    nc.scalar.activation(
                        _ap(om_fmb[0:HD, :], [(1, 512)], offset_elems=t * 512),
                        pt2[0:HD, :], AF.Copy)

                # att = sigmoid(om@gate) * tanh(om@fc); h_new += att
                # (two blocks per iteration -> half the ACT op count)
                for i in range(NBO // 2):
                    fg2 = ps_fg.tile([128, 2, 2 * BSO], f32, tag="fg2",
                                     name="fg2")
                    for j in range(2):
                        nc.tensor.matmul(fg2[:, j, :], om_fmb[:, 2 * i + j, :],
                                         wfg_sb, start=True, stop=True)
                    t_t = gru3.tile([128, 2, BSO], f32, tag="d_sb")
                    nc.scalar.activation(
                        t_t, _ap(fg2, [(2 * BSO, 2), (1, BSO)]), AF.Tanh)
                    t_s = gru3.tile([128, 2, BSO], f32, tag="zd")
                    nc.scalar.activation(
                        t_s, _ap(fg2, [(2 * BSO, 2), (1, BSO)],
                                 offset_elems=BSO), AF.Sigmoid)
                    nc.vector.tensor_mul(t_t, t_s, t_t)
                    asl = slice(2 * i * BSO, (2 * i + 2) * BSO)
                    nc.gpsimd.tensor_add(
                        h_new[:, asl], h_new[:, asl], _ap(t_t, [(1, 2 * BSO)]))

                yield
                # ---- masked blends (in-place over hx_bm/cx_bm) + stores ----
                # HW CopyPredicated wants an integer mask dtype
                mw_u8 = work.tile([128, NBO, BSO], mybir.dt.uint8, tag="mwu8")
                nc.gpsimd.tensor_copy(out=mw_u8, in_=_ap(mask, [(1, NBO), (0, BSO)]))
                mw_u8f = _ap(mw_u8, [(1, NHID)])
                nc.vector.copy_predicated(out=hx_bm[:], mask=mw_u8f, data=h_new[:])
                nc.vector.copy_predicated(out=cx_bm[:], mask=mw_u8f, data=h_new[:])
                nc.sync.dma_start(out=d_hxo[rows, :], in_=hx_bm)
                nc.sync.dma_start(out=d_cxo[rows, :], in_=cx_bm)
                # mask_w: materialize f32 broadcast in SBUF (HW DGE rejects a
                # step-0 fastest-moving dim), then plain store
                mw_sb = work.tile([128, NBO, BSO], f32, tag="mw")
                nc.gpsimd.tensor_copy(out=mw_sb,
                                      in_=_ap(mask, [(1, NBO), (0, BSO)]))
                nc.sync.dma_start(out=d_mw[rows, :], in_=_ap(mw_sb, [(1, NHID)]))

            # 1-group software-pipeline skew with fine-grained interleave:
            # front(g) segments are emitted alternating with back(g-1)
            # segments so every engine's in-order stream mixes PE-heavy and
            # DVE-heavy work at sub-group granularity.
            prev_st = None
            for g in range(NG):
                st = {}
                f = front(g, st)
                b = back(prev_st) if prev_st is not None else None
                for _ in f:
                    if b is not None:
                        next(b, None)
                if b is not None:
                    for _ in b:
                        pass
                prev_st = st
            for _ in back(prev_st):
                pass

    nc.compile()
    _CACHE["nc"] = nc
    return nc


def fold_weights(I):
    """Host-side weight folding (float64 for fidelity, cast down at the end)."""
    Wih = np.asarray(I["Wih"], np.float64)          # (8, 768, 1024)
    Wih_cat = Wih.transpose(2, 0, 1).reshape(1024, NBO * G3)
    W3 = (np.asarray(I["Wv_i"], np.float64)[1] @
          np.asarray(I["fc_i_w"], np.float64) @ Wih_cat)          # (512, 6144)
    WhhT = np.asarray(I["Whh"], np.float64).transpose(0, 2, 1)    # (8, 256, 768)
    # mha qkv concat; fold 1/sqrt(DK_M) into Wq_m
    Wq_m = np.asarray(I["Wq_m"], np.float64) / np.sqrt(DK_M)
    wqkv = np.concatenate(
        [Wq_m, np.asarray(I["Wk_m"], np.float64),
         np.asarray(I["Wv_m"], np.float64)], axis=2)              # (8, 256, 192)
    wfg = np.concatenate(
        [np.asarray(I["fc_m_w"], np.float64),
         np.asarray(I["gate_m_w"], np.float64)], axis=1)          # (64, 512)
    wq = np.asarray(I["Wq_i"], np.float64) / np.sqrt(DK_I)        # (8, 256, 64)
    wk1 = np.asarray(I["Wk_i"], np.float64)[1]                    # (512, 64)

    for name in ("fc_i_b", "bih", "bhh", "fc_m_b", "gate_m_b"):
        if np.any(np.asarray(I[name])):
            raise NotImplementedError(f"nonzero bias {name} not supported")

    import ml_dtypes
    tobf = lambda a: np.ascontiguousarray(a).astype(ml_dtypes.bfloat16)
    # SBUF-ready layouts: feature axis split into 128-partition chunks
    w3_l = W3.reshape(4, 128, NBO * G3).transpose(1, 0, 2)
    whh_l = WhhT.reshape(NBO, 2, 128, G3).transpose(2, 1, 0, 3)
    wqkv_l = wqkv.reshape(NBO, 2, 128, 3 * HD).transpose(2, 1, 0, 3)
    wq_l = wq.transpose(2, 0, 1)          # (64, 8, 256): u_n = Wq_n @ k1
    wk1_l = wk1.reshape(4, 128, DK_I).transpose(1, 0, 2)
    return {
        "w3": tobf(w3_l), "whh": tobf(whh_l), "wqkv": tobf(wqkv_l),
        "wfg": tobf(wfg),
        "wq": np.ascontiguousarray(wq_l.astype(np.float32)),
        "wk1": np.ascontiguousarray(wk1_l.astype(np.float32)),
    }


def core_input_maps(inputs):
    """Split full inputs into per-core in_maps."""
    w = fold_weights(inputs)
    inp = np.ascontiguousarray(np.asarray(inputs["inp"], np.float32))
    hx = np.ascontiguousarray(np.asarray(inputs["hx"], np.float32))
    cx = np.ascontiguousarray(np.asarray(inputs["cx"], np.float32))
    maps = []
    for c in range(N_CORES):
        rows = slice(c * B, (c + 1) * B)
        maps.append({"inp": inp[rows], "hx": hx[rows], "cx": cx[rows], **w})
    return maps


def kernel(**inputs):
    global last_results
    from concourse.bass_utils import run_bass_kernel_spmd

    nc = build_program()
    in_maps = core_input_maps(inputs)
    last_results = run_bass_kernel_spmd(
        nc, in_maps, list(range(N_CORES)),
        trace=bool(os.environ.get("BASS_TRACE")))
    res = last_results.results
    hx_out = np.concatenate([res[c]["hx_out"] for c in range(N_CORES)], axis=0)
    cx_out = np.concatenate([res[c]["cx_out"] for c in range(N_CORES)], axis=0)
    mask_w = np.concatenate([res[c]["mask_w"] for c in range(N_CORES)], axis=0)
    return hx_out, cx_out, mask_w
